# revision 8
# baseline (speedup 1.0000x reference)
"""LSTMCell (B=16384, IN=HID=512) on 8 TRN2 NeuronCores — v3.

Data-parallel over batch (2048 rows/core), weights replicated.
bf16 GEMM (the accuracy-safe PE floor: fp8 DoubleRow streams at 1
cycle/row on TRN2 hw, so a 3-pass fp8 split is 1.5x bf16 work).

vs the v1 baseline:
  - k-chunked DMA (128KB granularity) so the first matmul starts ~2-3us
    after launch instead of ~12us.
  - matmul loop is (r, g, k, nb): 4 batch-chunks stream per weight
    block, and redundant LDWEIGHTS are deleted post-schedule (verified
    on hw: the PE weight buffer persists across matmuls).
  - c_in / gates / cell-update / outputs in bf16: halves DMA traffic
    and doubles DVE throughput; everything stays hidden under the PE.
"""

import sys

sys.path.insert(0, "/opt/trn_rl_repo")

from contextlib import ExitStack

import ml_dtypes
import numpy as np

import concourse.bass as bass  # noqa: F401
import concourse.mybir as mybir
import concourse.tile as tile
from concourse import bacc
from concourse.bass_utils import run_bass_kernel_spmd

B_FULL, IN, HID = 16384, 512, 512
NCORES = 8
BL = B_FULL // NCORES  # 2048 batch rows per core
JW = 512               # batch columns per chunk (matmul free dim)
P = 128

BF16 = mybir.dt.bfloat16
F32 = mybir.dt.float32
AF = mybir.ActivationFunctionType
BF16_NP = ml_dtypes.bfloat16

NK = (IN + HID) // P   # 8  k-chunks of the contraction dim
NR = HID // P          # 4  row-blocks of H per gate
NM = 4 * HID // P      # 16 gate-row blocks total (i,g,f,o order)

WARMUP_MM = 20


def dedup_ldweights(nc):
    """Delete InstLdweights whose weights AP matches the immediately
    preceding LDWEIGHTS on the PE queue. Non-self-loading matmuls keep
    using the loaded weights (verified on hw). Deps of the removed LW
    are merged into the next PE instruction."""
    removed = 0
    for f in nc.m.functions:
        for b in f.blocks:
            insts = b.instructions
            last_key = None
            to_remove = []
            for idx, inst in enumerate(insts):
                if type(inst).__name__ == "InstLdweights":
                    key = str(inst.ins[0])
                    if key == last_key:
                        to_remove.append(idx)
                    last_key = key
            for idx in reversed(to_remove):
                lw = insts[idx]
                nxt = None
                for j in range(idx + 1, len(insts)):
                    if insts[j].engine == lw.engine:
                        nxt = insts[j]
                        break
                if nxt is not None:
                    nxt.merge_dependencies_from(lw)
                insts.remove(lw)
                removed += 1
    return removed


def build_nc(bl=BL):
    """Build the single-core Bass program (SPMD-replicated across cores)."""
    nbn = bl // JW
    nc = bacc.Bacc("TRN2", target_bir_lowering=False, debug=False)

    xh_in = nc.dram_tensor("xh_in", [NK, nbn, P, JW], BF16, kind="ExternalInput")
    wt_in = nc.dram_tensor("wt_in", [NK, P, 4 * HID], BF16, kind="ExternalInput")
    bias_in = nc.dram_tensor("bias_in", [P, NM], F32, kind="ExternalInput")
    c_in = nc.dram_tensor("c_in", [nbn, NR, P, JW], BF16, kind="ExternalInput")
    h_out = nc.dram_tensor("h_out", [nbn, NR, P, JW], BF16, kind="ExternalOutput")
    c_out = nc.dram_tensor("c_out", [nbn, NR, P, JW], BF16, kind="ExternalOutput")

    with ExitStack() as ctx:
        tc = ctx.enter_context(tile.TileContext(nc))
        wpool = ctx.enter_context(tc.tile_pool(name="w", bufs=1))
        xpool = ctx.enter_context(tc.tile_pool(name="xh", bufs=1))
        cpool = ctx.enter_context(tc.tile_pool(name="cin", bufs=1))
        gpool = ctx.enter_context(tc.tile_pool(name="gates", bufs=2))
        opool = ctx.enter_context(tc.tile_pool(name="outs", bufs=3))
        pspool = ctx.enter_context(tc.tile_pool(name="ps", bufs=1, space="PSUM"))

        # PE HAM warmup: keep the PE activity monitor busy through the
        # initial DMA/runtime window so real matmuls run at full clock.
        wu = wpool.tile([P, JW], BF16, tag="wu", name="wu")
        nc.vector.memset(wu[:], 0.0)
        wu_ps = pspool.tile([P, JW], F32, tag="ps0_0", name="wu_ps")
        for i in range(WARMUP_MM):
            # alternate weight slices so dedup keeps LW+MM pairs (more
            # realistic PE occupancy during ramp)
            nc.tensor.matmul(wu_ps[:], wu[:, (i % 2) * P : (i % 2 + 1) * P],
                             wu[:], start=True, stop=True)

        # Input DMAs, k-major so the first (g, k) matmul group can start
        # as soon as the first 128KB chunks land.
        #   gpsimd queue: weights (k-slices) + bias, then c_in
        #   sync queue:   xh chunks (k-major)
        #   scalar queue: outputs
        wts = []
        for k in range(NK):
            wt = wpool.tile([P, 4 * HID], BF16, tag=f"w{k}", name=f"w{k}")
            nc.gpsimd.dma_start(wt[:], wt_in[k])
            wts.append(wt)
        bias_t = wpool.tile([P, NM], F32, tag="bias", name="bias")
        nc.gpsimd.dma_start(bias_t[:], bias_in[:])

        xh = [[None] * NK for _ in range(nbn)]
        for k in range(NK):
            for nb in range(nbn):
                xt = xpool.tile([P, JW], BF16, tag=f"xh{nb}_{k}", name=f"xh{nb}_{k}")
                nc.sync.dma_start(xt[:], xh_in[k, nb])
                xh[nb][k] = xt

        cts = [[None] * NR for _ in range(nbn)]
        for nb in range(nbn):
            for r in range(NR):
                ct = cpool.tile([P, JW], BF16, tag=f"c{nb}_{r}", name=f"c{nb}_{r}")
                nc.gpsimd.dma_start(ct[:], c_in[nb, r])
                cts[nb][r] = ct

        for r in range(NR):
            gates = [[None] * nbn for _ in range(4)]
            for g in range(4):
                m = g * NR + r
                ms = slice(m * P, (m + 1) * P)
                ps = [
                    pspool.tile([P, JW], F32, tag=f"ps{g % 2}_{nb}",
                                name=f"ps{g % 2}_{nb}")
                    for nb in range(nbn)
                ]
                for k in range(NK):
                    for nb in range(nbn):
                        nc.tensor.matmul(
                            ps[nb][:],
                            wts[k][:, ms],
                            xh[nb][k][:],
                            start=(k == 0),
                            stop=(k == NK - 1),
                        )
                func = AF.Tanh if g == 1 else AF.Sigmoid
                for nb in range(nbn):
                    gt = gpool.tile([P, JW], F32, tag=f"g{g}_{nb}")
                    nc.scalar.activation(
                        gt[:], ps[nb][:], func, bias=bias_t[:, m : m + 1]
                    )
                    gates[g][nb] = gt
            for nb in range(nbn):
                it, gt, ft, ot = (gates[gg][nb] for gg in range(4))
                t1 = gpool.tile([P, JW], F32, tag="t1")
                t2 = gpool.tile([P, JW], F32, tag="t2")
                cn = opool.tile([P, JW], BF16, tag="cn")
                tch = gpool.tile([P, JW], BF16, tag="tch")
                hn = opool.tile([P, JW], BF16, tag="hn")
                nc.vector.tensor_mul(t1[:], it[:], gt[:])
                nc.vector.tensor_mul(t2[:], ft[:], cts[nb][r][:])
                nc.vector.tensor_add(cn[:], t1[:], t2[:])
                nc.scalar.activation(tch[:], cn[:], AF.Tanh)
                nc.vector.tensor_mul(hn[:], ot[:], tch[:])
                nc.scalar.dma_start(c_out[nb, r], cn[:])
                nc.scalar.dma_start(h_out[nb, r], hn[:])
    dedup_ldweights(nc)
    nc.compile()
    return nc


def prep_shared(Wxi, Wxg, Wxf, Wxo, Whi, Whg, Whf, Who, bias_sum):
    """wt_in [NK,P,4H] bf16 and bias_in [P,NM] f32 (gate order i,g,f,o)."""
    Wx = np.concatenate([Wxi, Wxg, Wxf, Wxo], axis=0)  # [4H, IN]
    Wh = np.concatenate([Whi, Whg, Whf, Who], axis=0)  # [4H, HID]
    WT = np.concatenate([Wx.T, Wh.T], axis=0)          # [K=1024, 4H]
    wt_arr = np.ascontiguousarray(
        WT.reshape(NK, P, 4 * HID).astype(BF16_NP)
    )
    bias_arr = np.ascontiguousarray(
        bias_sum.reshape(NM, P).T.astype(np.float32)
    )
    return wt_arr, bias_arr


def prep_core(x_s, h_s, c_s):
    """Per-core xh_in [NK,nb,P,JW] bf16 and c_in [nb,NR,P,JW] bf16."""
    bl = x_s.shape[0]
    nbn = bl // JW
    xhT = np.concatenate([x_s, h_s], axis=1).T  # [K=1024, bl]
    xh_arr = np.ascontiguousarray(
        xhT.reshape(NK, P, nbn, JW).transpose(0, 2, 1, 3).astype(BF16_NP)
    )
    cT = c_s.T  # [HID, bl]
    c_arr = np.ascontiguousarray(
        cT.reshape(NR, P, nbn, JW).transpose(2, 0, 1, 3).astype(BF16_NP)
    )
    return xh_arr, c_arr


def post_core(arr):
    """[nb,NR,P,JW] -> [bl, HID] f32"""
    arr = np.asarray(arr).astype(np.float32)
    nbn = arr.size // (NR * P * JW)
    arr = arr.reshape(nbn, NR, P, JW)
    return arr.transpose(0, 3, 1, 2).reshape(nbn * JW, HID)


_NC_CACHE = {}


def _get_nc(bl=BL):
    if bl not in _NC_CACHE:
        _NC_CACHE[bl] = build_nc(bl)
    return _NC_CACHE[bl]


def make_in_maps(x, h, c, Wxi, bxi, Wxo, bxo, Wxf, bxf, Wxg, bxg,
                 Whi, bhi, Who, bho, Whf, bhf, Whg, bhg, ncores=NCORES):
    bias_sum = np.concatenate(
        [bxi + bhi, bxg + bhg, bxf + bhf, bxo + bho], axis=0
    ).astype(np.float32)
    wt_arr, bias_arr = prep_shared(Wxi, Wxg, Wxf, Wxo, Whi, Whg, Whf, Who, bias_sum)
    bl = x.shape[0] // ncores
    in_maps = []
    for i in range(ncores):
        s = slice(i * bl, (i + 1) * bl)
        xh_arr, c_arr = prep_core(
            np.asarray(x[s], np.float32),
            np.asarray(h[s], np.float32),
            np.asarray(c[s], np.float32),
        )
        in_maps.append(
            {"xh_in": xh_arr, "wt_in": wt_arr, "bias_in": bias_arr, "c_in": c_arr}
        )
    return in_maps


def kernel(x, h, c, Wxi, bxi, Wxo, bxo, Wxf, bxf, Wxg, bxg,
           Whi, bhi, Who, bho, Whf, bhf, Whg, bhg):
    args = dict(
        x=np.asarray(x, np.float32), h=np.asarray(h, np.float32),
        c=np.asarray(c, np.float32),
        Wxi=np.asarray(Wxi, np.float32), bxi=np.asarray(bxi, np.float32),
        Wxo=np.asarray(Wxo, np.float32), bxo=np.asarray(bxo, np.float32),
        Wxf=np.asarray(Wxf, np.float32), bxf=np.asarray(bxf, np.float32),
        Wxg=np.asarray(Wxg, np.float32), bxg=np.asarray(bxg, np.float32),
        Whi=np.asarray(Whi, np.float32), bhi=np.asarray(bhi, np.float32),
        Who=np.asarray(Who, np.float32), bho=np.asarray(bho, np.float32),
        Whf=np.asarray(Whf, np.float32), bhf=np.asarray(bhf, np.float32),
        Whg=np.asarray(Whg, np.float32), bhg=np.asarray(bhg, np.float32),
    )
    in_maps = make_in_maps(**args)
    nc = _get_nc(BL)
    res = run_bass_kernel_spmd(nc, in_maps, core_ids=list(range(NCORES)))
    h_new = np.empty((B_FULL, HID), np.float32)
    c_new = np.empty((B_FULL, HID), np.float32)
    for i in range(NCORES):
        s = slice(i * BL, (i + 1) * BL)
        h_new[s] = post_core(res.results[i]["h_out"])
        c_new[s] = post_core(res.results[i]["c_out"])
    return (h_new, c_new)


# revision 10
# speedup vs baseline: 1.2103x; 1.2103x over previous
"""LSTMCell (B=16384, IN=HID=512) on 8 TRN2 NeuronCores — v3.

Data-parallel over batch (2048 rows/core), weights replicated.
bf16 GEMM (the accuracy-safe PE floor: fp8 DoubleRow streams at 1
cycle/row on TRN2 hw, so a 3-pass fp8 split is 1.5x bf16 work).

vs the v1 baseline:
  - k-chunked DMA (128KB granularity) so the first matmul starts ~2-3us
    after launch instead of ~12us.
  - matmul loop is (r, g, k, nb): 4 batch-chunks stream per weight
    block, and redundant LDWEIGHTS are deleted post-schedule (verified
    on hw: the PE weight buffer persists across matmuls).
  - c_in / gates / cell-update / outputs in bf16: halves DMA traffic
    and doubles DVE throughput; everything stays hidden under the PE.
"""

import sys

sys.path.insert(0, "/opt/trn_rl_repo")

from contextlib import ExitStack

import ml_dtypes
import numpy as np

import concourse.bass as bass  # noqa: F401
import concourse.mybir as mybir
import concourse.tile as tile
from concourse import bacc
from concourse.bass_utils import run_bass_kernel_spmd

B_FULL, IN, HID = 16384, 512, 512
NCORES = 8
BL = B_FULL // NCORES  # 2048 batch rows per core
JW = 512               # batch columns per chunk (matmul free dim)
P = 128

BF16 = mybir.dt.bfloat16
F32 = mybir.dt.float32
AF = mybir.ActivationFunctionType
BF16_NP = ml_dtypes.bfloat16

NK = (IN + HID) // P   # 8  k-chunks of the contraction dim
NR = HID // P          # 4  row-blocks of H per gate
NM = 4 * HID // P      # 16 gate-row blocks total (i,g,f,o order)

WARMUP_MM = 20


def dedup_ldweights(nc):
    """Delete InstLdweights whose weights AP matches the immediately
    preceding LDWEIGHTS on the PE queue. Non-self-loading matmuls keep
    using the loaded weights (verified on hw). Deps of the removed LW
    are merged into the next PE instruction."""
    removed = 0
    for f in nc.m.functions:
        for b in f.blocks:
            insts = b.instructions
            last_key = None
            to_remove = []
            for idx, inst in enumerate(insts):
                if type(inst).__name__ == "InstLdweights":
                    key = str(inst.ins[0])
                    if key == last_key:
                        to_remove.append(idx)
                    last_key = key
            for idx in reversed(to_remove):
                lw = insts[idx]
                nxt = None
                for j in range(idx + 1, len(insts)):
                    if insts[j].engine == lw.engine:
                        nxt = insts[j]
                        break
                if nxt is not None:
                    nxt.merge_dependencies_from(lw)
                insts.remove(lw)
                removed += 1
    return removed


def build_nc(bl=BL):
    """Build the single-core Bass program (SPMD-replicated across cores)."""
    nbn = bl // JW
    nc = bacc.Bacc("TRN2", target_bir_lowering=False, debug=False)

    xh_in = nc.dram_tensor("xh_in", [NK, P, nbn, JW], BF16, kind="ExternalInput")
    wt_in = nc.dram_tensor("wt_in", [NK, P, 4 * HID], BF16, kind="ExternalInput")
    bias_in = nc.dram_tensor("bias_in", [P, NM], F32, kind="ExternalInput")
    c_in = nc.dram_tensor("c_in", [nbn, P, NR, JW], BF16, kind="ExternalInput")
    h_out = nc.dram_tensor("h_out", [nbn, NR, P, JW], BF16, kind="ExternalOutput")
    c_out = nc.dram_tensor("c_out", [nbn, NR, P, JW], BF16, kind="ExternalOutput")

    with ExitStack() as ctx:
        tc = ctx.enter_context(tile.TileContext(nc))
        wpool = ctx.enter_context(tc.tile_pool(name="w", bufs=1))
        xpool = ctx.enter_context(tc.tile_pool(name="xh", bufs=1))
        cpool = ctx.enter_context(tc.tile_pool(name="cin", bufs=1))
        gpool = ctx.enter_context(tc.tile_pool(name="gates", bufs=2))
        opool = ctx.enter_context(tc.tile_pool(name="outs", bufs=3))
        pspool = ctx.enter_context(tc.tile_pool(name="ps", bufs=1, space="PSUM"))

        # PE HAM warmup: keep the PE activity monitor busy through the
        # initial DMA/runtime window so real matmuls run at full clock.
        wu = wpool.tile([P, JW], BF16, tag="wu", name="wu")
        nc.vector.memset(wu[:], 0.0)
        wu_ps = pspool.tile([P, JW], F32, tag="ps0_0", name="wu_ps")
        for i in range(WARMUP_MM):
            # alternate weight slices so dedup keeps LW+MM pairs (more
            # realistic PE occupancy during ramp)
            nc.tensor.matmul(wu_ps[:], wu[:, (i % 2) * P : (i % 2 + 1) * P],
                             wu[:], start=True, stop=True)

        # Input DMAs, k-major so the first (g, k) matmul group can start
        # as soon as the first 128KB chunks land.
        #   gpsimd queue: weights (k-slices) + bias, then c_in
        #   sync queue:   xh chunks (k-major)
        #   scalar queue: outputs
        wts = []
        for k in range(NK):
            wt = wpool.tile([P, 4 * HID], BF16, tag=f"w{k}", name=f"w{k}")
            nc.gpsimd.dma_start(wt[:], wt_in[k])
            wts.append(wt)
        bias_t = wpool.tile([P, NM], F32, tag="bias", name="bias")
        nc.gpsimd.dma_start(bias_t[:], bias_in[:])

        xh_big = xpool.tile([P, NK, nbn, JW], BF16, tag="xh", name="xh")
        for k in range(NK):
            nc.sync.dma_start(xh_big[:, k], xh_in[k])

        c_big = cpool.tile([P, nbn, NR, JW], BF16, tag="c", name="c")
        for nb in range(nbn):
            nc.gpsimd.dma_start(c_big[:, nb], c_in[nb])

        for r in range(NR):
            gates = [[None] * nbn for _ in range(4)]
            for g in range(4):
                m = g * NR + r
                ms = slice(m * P, (m + 1) * P)
                ps = [
                    pspool.tile([P, JW], F32, tag=f"ps{g % 2}_{nb}",
                                name=f"ps{g % 2}_{nb}")
                    for nb in range(nbn)
                ]
                for k in range(NK):
                    for nb in range(nbn):
                        nc.tensor.matmul(
                            ps[nb][:],
                            wts[k][:, ms],
                            xh_big[:, k, nb, :],
                            start=(k == 0),
                            stop=(k == NK - 1),
                        )
                func = AF.Tanh if g == 1 else AF.Sigmoid
                for nb in range(nbn):
                    gt = gpool.tile([P, JW], F32, tag=f"g{g}_{nb}")
                    nc.scalar.activation(
                        gt[:], ps[nb][:], func, bias=bias_t[:, m : m + 1]
                    )
                    gates[g][nb] = gt
            for nb in range(nbn):
                it, gt, ft, ot = (gates[gg][nb] for gg in range(4))
                t1 = gpool.tile([P, JW], F32, tag="t1")
                t2 = gpool.tile([P, JW], F32, tag="t2")
                cn = opool.tile([P, JW], BF16, tag="cn")
                tch = gpool.tile([P, JW], BF16, tag="tch")
                hn = opool.tile([P, JW], BF16, tag="hn")
                nc.vector.tensor_mul(t1[:], it[:], gt[:])
                nc.vector.tensor_mul(t2[:], ft[:], c_big[:, nb, r, :])
                nc.vector.tensor_add(cn[:], t1[:], t2[:])
                nc.scalar.activation(tch[:], cn[:], AF.Tanh)
                nc.vector.tensor_mul(hn[:], ot[:], tch[:])
                nc.scalar.dma_start(c_out[nb, r], cn[:])
                nc.scalar.dma_start(h_out[nb, r], hn[:])
    dedup_ldweights(nc)
    nc.compile()
    return nc


def prep_shared(Wxi, Wxg, Wxf, Wxo, Whi, Whg, Whf, Who, bias_sum):
    """wt_in [NK,P,4H] bf16 and bias_in [P,NM] f32 (gate order i,g,f,o)."""
    Wx = np.concatenate([Wxi, Wxg, Wxf, Wxo], axis=0)  # [4H, IN]
    Wh = np.concatenate([Whi, Whg, Whf, Who], axis=0)  # [4H, HID]
    WT = np.concatenate([Wx.T, Wh.T], axis=0)          # [K=1024, 4H]
    wt_arr = np.ascontiguousarray(
        WT.reshape(NK, P, 4 * HID).astype(BF16_NP)
    )
    bias_arr = np.ascontiguousarray(
        bias_sum.reshape(NM, P).T.astype(np.float32)
    )
    return wt_arr, bias_arr


def prep_core(x_s, h_s, c_s):
    """Per-core xh_in [NK,nb,P,JW] bf16 and c_in [nb,NR,P,JW] bf16."""
    bl = x_s.shape[0]
    nbn = bl // JW
    xhT = np.concatenate([x_s, h_s], axis=1).T  # [K=1024, bl]
    xh_arr = np.ascontiguousarray(
        xhT.reshape(NK, P, nbn, JW).astype(BF16_NP)
    )
    cT = c_s.T  # [HID, bl]
    c_arr = np.ascontiguousarray(
        cT.reshape(NR, P, nbn, JW).transpose(2, 1, 0, 3).astype(BF16_NP)
    )
    return xh_arr, c_arr


def post_core(arr):
    """[nb,NR,P,JW] -> [bl, HID] f32"""
    arr = np.asarray(arr).astype(np.float32)
    nbn = arr.size // (NR * P * JW)
    arr = arr.reshape(nbn, NR, P, JW)
    return arr.transpose(0, 3, 1, 2).reshape(nbn * JW, HID)


_NC_CACHE = {}


def _get_nc(bl=BL):
    if bl not in _NC_CACHE:
        _NC_CACHE[bl] = build_nc(bl)
    return _NC_CACHE[bl]


def make_in_maps(x, h, c, Wxi, bxi, Wxo, bxo, Wxf, bxf, Wxg, bxg,
                 Whi, bhi, Who, bho, Whf, bhf, Whg, bhg, ncores=NCORES):
    bias_sum = np.concatenate(
        [bxi + bhi, bxg + bhg, bxf + bhf, bxo + bho], axis=0
    ).astype(np.float32)
    wt_arr, bias_arr = prep_shared(Wxi, Wxg, Wxf, Wxo, Whi, Whg, Whf, Who, bias_sum)
    bl = x.shape[0] // ncores
    in_maps = []
    for i in range(ncores):
        s = slice(i * bl, (i + 1) * bl)
        xh_arr, c_arr = prep_core(
            np.asarray(x[s], np.float32),
            np.asarray(h[s], np.float32),
            np.asarray(c[s], np.float32),
        )
        in_maps.append(
            {"xh_in": xh_arr, "wt_in": wt_arr, "bias_in": bias_arr, "c_in": c_arr}
        )
    return in_maps


def kernel(x, h, c, Wxi, bxi, Wxo, bxo, Wxf, bxf, Wxg, bxg,
           Whi, bhi, Who, bho, Whf, bhf, Whg, bhg):
    args = dict(
        x=np.asarray(x, np.float32), h=np.asarray(h, np.float32),
        c=np.asarray(c, np.float32),
        Wxi=np.asarray(Wxi, np.float32), bxi=np.asarray(bxi, np.float32),
        Wxo=np.asarray(Wxo, np.float32), bxo=np.asarray(bxo, np.float32),
        Wxf=np.asarray(Wxf, np.float32), bxf=np.asarray(bxf, np.float32),
        Wxg=np.asarray(Wxg, np.float32), bxg=np.asarray(bxg, np.float32),
        Whi=np.asarray(Whi, np.float32), bhi=np.asarray(bhi, np.float32),
        Who=np.asarray(Who, np.float32), bho=np.asarray(bho, np.float32),
        Whf=np.asarray(Whf, np.float32), bhf=np.asarray(bhf, np.float32),
        Whg=np.asarray(Whg, np.float32), bhg=np.asarray(bhg, np.float32),
    )
    in_maps = make_in_maps(**args)
    nc = _get_nc(BL)
    res = run_bass_kernel_spmd(nc, in_maps, core_ids=list(range(NCORES)))
    h_new = np.empty((B_FULL, HID), np.float32)
    c_new = np.empty((B_FULL, HID), np.float32)
    for i in range(NCORES):
        s = slice(i * BL, (i + 1) * BL)
        h_new[s] = post_core(res.results[i]["h_out"])
        c_new[s] = post_core(res.results[i]["c_out"])
    return (h_new, c_new)
